# revision 31
# baseline (speedup 1.0000x reference)
"""ContextualLoss forward on 8 Trainium2 NeuronCores (v2.3).

Math (reference):
    mu[m]   = mean_c Y[c, m]                      (PONO over channels of Y)
    Xc = X - mu ; Yc = Y - mu                     (both centered by Y's mean)
    cos[i,j] = <Xc_i, Yc_j> / (|Xc_i| |Yc_j|)
    d = 1 - cos ; dn = d / (min_j d + 1e-3) ; w = exp((1 - dn)/0.1)
    A = w / sum_j w ; CX_b = mean_i max_j A ; loss = mean_b -log CX_b

Device-side structure:
  * Inputs arrive bf16, laid out host-side so every DMA descriptor is a
    4-16KB contiguous run per partition.
  * mu/qy column stats via [128x128] all-ones stationary matmuls -> [128, Q]
    broadcasts land in PSUM directly.
  * 1/|Yc_j| = exp(-0.5*ln(qy)) on ScalarE; the activation-table picker is
    pinned so square/ln/exp/identity/copy all resolve to the
    natural_log_exp_and_others set (one table load for the whole kernel).
  * y_eff = (Y - mu) * inv_ny in SBUF (bf16) -> the PE matmul emits
    d_sb = cos/|Xc| rows; PSUM->SBUF moves are pure copies, split between
    DVE and ScalarE to balance the two hot engines.
  * Row max via DVE reduce_max per quarter (from SBUF d);
    w = Exp(scale_i*d + bias_i) with scale_i = 10*r*inv_nx, bias_i = 10-10r,
    r = 1/(dmin+1e-3); accum_out gives sum_j w;
    max_j A = exp(0.01*r)/sum_w analytically.

Sharding: core c -> sample b = c//2, row-half h = c%2 (2048 rows each).
Each core's Y is column-permuted host-side to [own-half | other-half] so the
identical SPMD program reads the X-half's means from columns [0, 2048).
"""

import sys

sys.path.insert(0, "/opt/trn_rl_repo")

import numpy as np

import concourse.bass as bass
import concourse.tile as tile
from concourse import bacc
from concourse import mybir
from concourse.bass_utils import run_bass_kernel_spmd

B = 4
C = 256
M = 4096  # 64*64 spatial positions
HALF = M // 2  # rows per core
NT = HALF // 128  # 16 i-tiles per core
N_CORES = 8
Q = 1024  # quarter width

F32 = mybir.dt.float32
BF16 = mybir.dt.bfloat16
AF = mybir.ActivationFunctionType
ALU = mybir.AluOpType

LN10 = float(np.log(10.0))


def _pin_act_table(arch: str):
    """Constrain the ACT table-set picker so every function this kernel uses
    (square/ln/exp/identity/copy) resolves to natural_log_exp_and_others.
    Only the cached *contents* used for placement decisions are edited; set
    ids keep their act_info.json indices, so codegen stays correct."""
    from concourse import hw_specs

    tables = hw_specs.get_activation_tables(arch)
    keep = "natural_log_exp_and_others"
    ours = {AF.Square, AF.Ln, AF.Exp, AF.Identity, AF.Copy}
    if keep not in tables:
        return
    for name, fns in tables.items():
        if name != keep:
            fns -= ours


def build_nc() -> bass.Bass:
    nc = bacc.Bacc()
    _pin_act_table(nc.m.arch)

    x_d = nc.declare_dram_parameter("x", [128, 2, HALF], BF16, isOutput=False)
    y_d = nc.declare_dram_parameter("y", [4, 128, 2, Q], BF16, isOutput=False)
    v_d = nc.declare_dram_parameter("v", [128, NT], F32, isOutput=True)

    with tile.TileContext(nc) as tc:
        with (
            tc.tile_pool(name="io", bufs=1) as io,
            tc.tile_pool(name="consts", bufs=1) as consts,
            tc.tile_pool(name="stats", bufs=1) as stats,
            tc.tile_pool(name="sqp", bufs=2) as sqp,
            tc.tile_pool(name="dpool", bufs=5) as dpool,
            tc.tile_pool(name="wpool", bufs=1) as wpool,
            tc.tile_pool(name="mains", bufs=6) as mains,
        ):
            y_bf = io.tile([128, 2, M], BF16)
            x_bf = io.tile([128, 2, HALF], BF16)
            inv_ny = io.tile([128, M], BF16)
            sq_x = io.tile([128, 2, HALF], BF16)

            ones_inv256 = consts.tile([128, 128], BF16)
            nc.vector.memset(ones_inv256, 1.0 / 256.0)
            ones128 = consts.tile([128, 128], BF16)
            nc.vector.memset(ones128, 1.0)
            ones_col_bf = consts.tile([128, 1], BF16)
            nc.vector.memset(ones_col_bf, 1.0)
            ln10_col = consts.tile([128, 1], F32)
            nc.vector.memset(ln10_col, LN10)

            inv_nx = stats.tile([128, NT], F32)
            inv_nx10 = stats.tile([128, NT], F32)
            neg_inv_nx = stats.tile([128, NT], F32)
            r16 = stats.tile([128, NT], F32)
            sumw16 = stats.tile([128, NT], F32)
            maxw16 = stats.tile([128, NT], F32)
            rs16 = stats.tile([128, NT], F32)
            v16 = stats.tile([128, NT], F32)

            def y_dma(q):
                nc.sync.dma_start(
                    out=y_bf[:, :, q * Q : (q + 1) * Q], in_=y_d[q, :, :, :]
                )

            def pre_y(q, pre):
                # mu broadcast: ones(1/256).T @ y -> [128, Q] in PSUM
                mu_ps = pre.tile([128, Q], F32, tag="pre")
                for k in range(2):
                    for j in range(2):
                        nc.tensor.matmul(
                            mu_ps[:, j * 512 : (j + 1) * 512],
                            lhsT=ones_inv256[:, :],
                            rhs=y_bf[:, k, q * Q + j * 512 : q * Q + (j + 1) * 512],
                            start=(k == 0),
                            stop=(k == 1),
                        )
                for k in range(2):
                    nc.vector.tensor_sub(
                        y_bf[:, k, q * Q : (q + 1) * Q],
                        y_bf[:, k, q * Q : (q + 1) * Q],
                        mu_ps[:, :],
                    )
                if q < 2:
                    # x columns [0, HALF) share these mu columns
                    for k in range(2):
                        nc.vector.tensor_sub(
                            x_bf[:, k, q * Q : (q + 1) * Q],
                            x_bf[:, k, q * Q : (q + 1) * Q],
                            mu_ps[:, :],
                        )
                # squares of centered y -> column sumsq in PSUM
                sq = sqp.tile([128, 2, Q], BF16, tag="sq")
                qy_ps = pre.tile([128, Q], F32, tag="pre")
                for k in range(2):
                    nc.scalar.activation(
                        sq[:, k, :], y_bf[:, k, q * Q : (q + 1) * Q], AF.Square
                    )
                    for j in range(2):
                        nc.tensor.matmul(
                            qy_ps[:, j * 512 : (j + 1) * 512],
                            lhsT=ones128[:, :],
                            rhs=sq[:, k, j * 512 : (j + 1) * 512],
                            start=(k == 0),
                            stop=(k == 1),
                        )
                # inv_ny = exp(-0.5 * ln(qy))
                lnq = sqp.tile([128, Q], F32, tag="lnq")
                nc.scalar.activation(lnq[:, :], qy_ps[:, :], AF.Ln)
                nc.scalar.activation(
                    inv_ny[:, q * Q : (q + 1) * Q], lnq[:, :], AF.Exp, scale=-0.5
                )
                # y_eff = centered y * inv_ny (bf16 2x)
                for k in range(2):
                    nc.vector.tensor_mul(
                        y_bf[:, k, q * Q : (q + 1) * Q],
                        y_bf[:, k, q * Q : (q + 1) * Q],
                        inv_ny[:, q * Q : (q + 1) * Q],
                    )

            def xstat(pre):
                # squares of centered x, then per-row-of-tile channel sums
                nc.scalar.activation(sq_x[:, :, :], x_bf[:, :, :], AF.Square)
                ps = pre.tile([128, NT], F32, tag="pre")
                for t in range(NT):
                    for k in range(2):
                        nc.tensor.matmul(
                            ps[:, t : t + 1],
                            lhsT=sq_x[:, k, t * 128 : (t + 1) * 128],
                            rhs=ones_col_bf[:, :],
                            start=(k == 0),
                            stop=(k == 1),
                        )
                # inv_nx = exp(-0.5 ln nx2); inv_nx10 folds the *10 as +ln10
                lnn = stats.tile([128, NT], F32)
                nc.scalar.activation(lnn[:, :], ps[:, :], AF.Ln)
                nc.scalar.activation(inv_nx[:, :], lnn[:, :], AF.Exp, scale=-0.5)
                nc.scalar.activation(
                    inv_nx10[:, :], lnn[:, :], AF.Exp, scale=-0.5, bias=ln10_col[:, :]
                )
                nc.vector.tensor_scalar_mul(neg_inv_nx[:, :], inv_nx[:, :], -1.0)

            def g_quarter(gp, t, q, d_sb, cmax4, on_dve):
                ps = gp.tile([128, Q], F32, tag="g")
                for k in range(2):
                    for j in range(2):
                        nc.tensor.matmul(
                            ps[:, j * 512 : (j + 1) * 512],
                            lhsT=x_bf[:, k, t * 128 : (t + 1) * 128],
                            rhs=y_bf[:, k, q * Q + j * 512 : q * Q + (j + 1) * 512],
                            start=(k == 0),
                            stop=(k == 1),
                        )
                if on_dve:
                    nc.vector.tensor_copy(d_sb[:, q * Q : (q + 1) * Q], ps[:, :])
                else:
                    nc.scalar.copy(d_sb[:, q * Q : (q + 1) * Q], ps[:, :])
                nc.vector.reduce_max(
                    cmax4[:, q : q + 1],
                    d_sb[:, q * Q : (q + 1) * Q],
                    axis=mybir.AxisListType.X,
                )

            def tile_stats(t, cmax4):
                cmax = mains.tile([128, 1], F32)
                u = mains.tile([128, 1], F32)
                scale_i = mains.tile([128, 1], F32, tag=f"scale{t % 3}")
                bias_i = mains.tile([128, 1], F32, tag=f"bias{t % 3}")
                nc.vector.reduce_max(cmax[:, :], cmax4[:, :], axis=mybir.AxisListType.X)
                # u = 1.001 - cmax * inv_nx
                nc.vector.tensor_scalar(
                    out=u[:, :],
                    in0=cmax[:, :],
                    scalar1=neg_inv_nx[:, t : t + 1],
                    scalar2=1.001,
                    op0=ALU.mult,
                    op1=ALU.add,
                )
                nc.vector.reciprocal(r16[:, t : t + 1], u[:, :])
                # scale_i = 10*r*inv_nx ; bias_i = 10 - 10*r (tiny DVE ops)
                nc.vector.tensor_mul(
                    scale_i[:, :], r16[:, t : t + 1], inv_nx10[:, t : t + 1]
                )
                nc.vector.tensor_scalar(
                    out=bias_i[:, :],
                    in0=r16[:, t : t + 1],
                    scalar1=-10.0,
                    scalar2=10.0,
                    op0=ALU.mult,
                    op1=ALU.add,
                )
                return scale_i, bias_i

            def tile_exp(t, d_sb, scale_i, bias_i):
                w_sb = wpool.tile([128, M], BF16)
                nc.scalar.activation(
                    out=w_sb[:, :],
                    in_=d_sb[:, :],
                    func=AF.Exp,
                    bias=bias_i[:, :],
                    scale=scale_i[:, :],
                    accum_out=sumw16[:, t : t + 1],
                )

            # DVE-copy share: ~24/64 quarters on DVE, rest on ScalarE
            def dve_copy(t, q):
                return (t * 4 + q) % 8 < 3

            # ---- schedule ------------------------------------------------
            with (
                tc.tile_pool(name="pre", bufs=2, space="PSUM") as pre,
                tc.tile_pool(name="gpa", bufs=2, space="PSUM") as gp_a,
            ):
                nc.sync.dma_start(out=y_bf[:, 0, 0:Q], in_=y_d[0, :, 0, :])
                nc.sync.dma_start(out=y_bf[:, 1, 0:Q], in_=y_d[0, :, 1, :])
                y_dma(1)
                nc.sync.dma_start(out=x_bf[:, :, :], in_=x_d[:, :, :])
                # HAM warmup: keep PE busy during the input DMA so the clock
                # gate opens before the real matmuls arrive
                warm = pre.tile([128, 128], F32, tag="pre")
                for _ in range(24):
                    nc.tensor.matmul(
                        warm[:, :], lhsT=ones128[:, :], rhs=ones128[:, :],
                        start=True, stop=True,
                    )
                pre_y(0, pre)
                pre_y(1, pre)
                y_dma(2)
                y_dma(3)
                # tile 0 overlaps preprocessing of quarters 2/3
                d_sb0 = dpool.tile([128, M], F32, tag="d_sb")
                cmax4_0 = mains.tile([128, 4], F32, tag="cmax4")
                g_quarter(gp_a, 0, 0, d_sb0, cmax4_0, dve_copy(0, 0))
                g_quarter(gp_a, 0, 1, d_sb0, cmax4_0, dve_copy(0, 1))
                pre_y(2, pre)
                pre_y(3, pre)
                xstat(pre)
                g_quarter(gp_a, 0, 2, d_sb0, cmax4_0, dve_copy(0, 2))
                g_quarter(gp_a, 0, 3, d_sb0, cmax4_0, dve_copy(0, 3))
                sb0 = tile_stats(0, cmax4_0)

            with tc.tile_pool(name="gpb", bufs=4, space="PSUM") as gp_b:
                tile_exp(0, d_sb0, *sb0)
                for t in range(1, NT):
                    d_sb = dpool.tile([128, M], F32, tag="d_sb")
                    cmax4 = mains.tile([128, 4], F32, tag="cmax4")
                    for q in range(4):
                        g_quarter(gp_b, t, q, d_sb, cmax4, dve_copy(t, q))
                    sb = tile_stats(t, cmax4)
                    tile_exp(t, d_sb, *sb)

            # ---- epilogue: v = exp(0.01*r) / sumw -----------------------
            nc.scalar.activation(maxw16[:, :], r16[:, :], AF.Exp, scale=0.01)
            nc.vector.reciprocal(rs16[:, :], sumw16[:, :])
            nc.vector.tensor_mul(v16[:, :], maxw16[:, :], rs16[:, :])
            nc.sync.dma_start(out=v_d[:, :], in_=v16[:, :])

    nc.compile()
    return nc


_NC = None


def _get_nc():
    global _NC
    if _NC is None:
        _NC = build_nc()
    return _NC


def make_in_maps(X, Y):
    """Per-core bf16 inputs. Y columns permuted to [own-half | other-half];
    layouts match the DMA-friendly dram shapes (x: [p,k,i], y: [q,p,k,j])."""
    import ml_dtypes

    bf16 = ml_dtypes.bfloat16
    in_maps = []
    for c in range(N_CORES):
        b, h = c // 2, c % 2
        xs = X[b][:, h * HALF : (h + 1) * HALF].astype(bf16)
        ys = np.concatenate(
            [
                Y[b][:, h * HALF : (h + 1) * HALF],
                Y[b][:, (1 - h) * HALF : (2 - h) * HALF],
            ],
            axis=1,
        ).astype(bf16)
        # channel c = k*128 + p
        xs = np.ascontiguousarray(xs.reshape(2, 128, HALF).transpose(1, 0, 2))
        ys = np.ascontiguousarray(ys.reshape(2, 128, 4, Q).transpose(2, 1, 0, 3))
        in_maps.append({"x": xs, "y": ys})
    return in_maps


def finish_host(results):
    """results: list of 8 per-core dicts with 'v' [128, NT]."""
    cx = np.zeros(B, dtype=np.float64)
    for c in range(N_CORES):
        cx[c // 2] += results[c]["v"].astype(np.float64).sum()
    cx /= M
    return np.float32(np.mean(-np.log(cx)))


def run(X_features, Y_features, trace=False, tmpdir=None):
    X = np.asarray(X_features, dtype=np.float32).reshape(B, C, M)
    Y = np.asarray(Y_features, dtype=np.float32).reshape(B, C, M)
    nc = _get_nc()
    res = run_bass_kernel_spmd(
        nc, make_in_maps(X, Y), list(range(N_CORES)), trace=trace, tmpdir=tmpdir
    )
    return finish_host(res.results), res


def kernel(X_features, Y_features):
    loss, _ = run(X_features, Y_features)
    return loss
